# revision 5
# baseline (speedup 1.0000x reference)
"""Multi-head attention (B=4, S=2048, D=1024, H=16) on 8 Trainium2 NeuronCores.

Sharding: (batch, head-group) tensor parallel — core c handles batch b = c//2
and head group g = c%2 (8 heads of 64 dims = 512 proj dims). Each core:
  qT/kT = Wg @ x.T         (projection, transposed layout [dims, tokens])
  v     = x @ Wv_g.T       (natural layout [tokens, dims], +ones column)
  per head: scoresT[k,q] = kh.T-tiles x qh-tiles; exp on ACT (scale=1/8 folded);
  ctxT[65,q] = [vh|1].T @ exp  (row 64 = softmax denominator, fused);
  normalize ctxT by reciprocal of row 64 (gpsimd partition-broadcast + DVE mul);
  outT_partial[1024, q] = woT_g.T @ ctxT.
Host sums the two head-group partials per batch and adds bo.

All matmuls run as float32r (full fp32 data, full PE rate at N>=512).
"""

import numpy as np

B, S, D, H, DK = 4, 2048, 1024, 16, 64
HPC = 8          # heads per core
GS = HPC * DK    # 512 projection dims per head group
N_CORES = 8
P = 128
SCALE = 0.125    # 1 / sqrt(DK)

# Optional knobs (used by test harness; inert for grading)
PROFILE_DIR = None     # if set, wrap device run with NTFF profile hook

_cache = {}


def _emit(ctx, tc, nc, t):
    import concourse.bass as bass
    from concourse import mybir

    f32 = mybir.dt.float32
    F32R = mybir.dt.float32r
    Exp = mybir.ActivationFunctionType.Exp

    def r(ap):
        return ap

    # ---- resident tiles --------------------------------------------------
    res = ctx.enter_context(tc.tile_pool(name="res", bufs=1))
    kT = [res.tile([P, S], F32R, tag=f"kT{i}", name=f"kT{i}") for i in range(4)]
    qT = [res.tile([P, S], F32R, tag=f"qT{i}", name=f"qT{i}") for i in range(4)]
    v = [res.tile([P, HPC, DK + 1], F32R, tag=f"v{i}", name=f"v{i}") for i in range(16)]
    cx = [res.tile([P, S], F32R, tag=f"cx{i}", name=f"cx{i}") for i in range(4)]

    bias = ctx.enter_context(tc.tile_pool(name="bias", bufs=1))
    bq_sb = bias.tile([P, 4], f32, tag="bq", name="bq")
    nc.sync.dma_start(out=bq_sb, in_=t["bq"].rearrange("(m p) -> p m", p=P))
    bk_sb = bias.tile([P, 4], f32, tag="bk", name="bk")
    nc.sync.dma_start(out=bk_sb, in_=t["bk"].rearrange("(m p) -> p m", p=P))
    bvb_sb = bias.tile([P, GS], f32, tag="bvb", name="bvb")
    nc.sync.dma_start(out=bvb_sb, in_=t["bvb"])

    # ones column for the fused softmax-denominator row of the AV matmul
    # (memset can't target fp32r; memset an f32 staging tile and DVE-copy,
    # which performs the fp32r rounding on write)
    ones_sb = bias.tile([P, HPC, 1], f32, tag="ones", name="ones")
    nc.vector.memset(ones_sb, 1.0)
    for i in range(16):
        nc.vector.tensor_copy(out=v[i][:, :, DK:DK + 1], in_=ones_sb)

    # ---- phase 1: projections -------------------------------------------
    def proj_T(xT_dram, w_dram, b_sb, out_tiles):
        # out[m][:, chunk] = (W.T).T @ x.T  -> [128 outdims, 512 tokens]
        with tc.tile_pool(name="w", bufs=1) as wp, \
             tc.tile_pool(name="xc", bufs=2) as xp, \
             tc.tile_pool(name="pp", bufs=2, space="PSUM") as pp:
            w_sb = wp.tile([P, 8, GS], F32R, tag="w", name="w")
            nc.sync.dma_start(out=w_sb, in_=w_dram.rearrange("(k p) m -> p k m", p=P))
            for c in range(4):
                xt = xp.tile([P, 8, 512], F32R, tag="x", name="x")
                nc.sync.dma_start(
                    out=xt,
                    in_=xT_dram[:, c * 512:(c + 1) * 512].rearrange(
                        "(k p) n -> p k n", p=P),
                )
                for m in range(4):
                    ps = pp.tile([P, 512], f32, tag="ps", name="ps")
                    for k in range(8):
                        nc.tensor.matmul(
                            ps,
                            lhsT=r(w_sb[:, k, m * P:(m + 1) * P]),
                            rhs=r(xt[:, k, :]),
                            start=(k == 0), stop=(k == 7),
                        )
                    nc.vector.tensor_scalar_add(
                        out=out_tiles[m][:, c * 512:(c + 1) * 512],
                        in0=ps, scalar1=b_sb[:, m:m + 1])

    def proj_V():
        with tc.tile_pool(name="w", bufs=1) as wp, \
             tc.tile_pool(name="xc", bufs=2) as xp, \
             tc.tile_pool(name="pp", bufs=2, space="PSUM") as pp:
            wv_sb = wp.tile([P, 8, GS], F32R, tag="w", name="w")
            nc.sync.dma_start(out=wv_sb,
                              in_=t["wvT"].rearrange("(k p) m -> p k m", p=P))
            bvb_h = bvb_sb.rearrange("p (h d) -> p h d", h=HPC)
            for c in range(4):
                xt = xp.tile([P, 8, 512], F32R, tag="x", name="x")
                nc.sync.dma_start(
                    out=xt,
                    in_=t["xvT"][:, c * 512:(c + 1) * 512].rearrange(
                        "(k p) n -> p k n", p=P),
                )
                for tt in range(4):
                    ps = pp.tile([P, GS], f32, tag="ps", name="ps")
                    for k in range(8):
                        nc.tensor.matmul(
                            ps,
                            lhsT=r(xt[:, k, tt * P:(tt + 1) * P]),
                            rhs=r(wv_sb[:, k, :]),
                            start=(k == 0), stop=(k == 7),
                        )
                    nc.vector.tensor_add(
                        out=v[c * 4 + tt][:, :, 0:DK],
                        in0=ps.rearrange("p (h d) -> p h d", h=HPC),
                        in1=bvb_h)

    proj_T(t["xkT"], t["wkT"], bk_sb, kT)
    proj_T(t["xqT"], t["wqT"], bq_sb, qT)
    proj_V()

    # ---- phases 2+3: attention + output projection ----------------------
    with tc.tile_pool(name="wo", bufs=1) as wop, \
         tc.tile_pool(name="pps", bufs=2, space="PSUM") as pps, \
         tc.tile_pool(name="pcx", bufs=1, space="PSUM") as pcx, \
         tc.tile_pool(name="expp", bufs=3) as expp, \
         tc.tile_pool(name="norm", bufs=2) as pn, \
         tc.tile_pool(name="osb", bufs=3) as posb:
        wo_sb = wop.tile([P, 4, D], F32R, tag="wo", name="wo")
        nc.sync.dma_start(out=wo_sb, in_=t["woT"].rearrange("(c p) m -> p c m", p=P))

        for qh in range(2):
            q0 = qh * 1024
            for hp in range(4):
                cA = pcx.tile([DK + 1, 1024], f32, tag="cA", name="cA")
                cB = pcx.tile([DK + 1, 1024], f32, tag="cB", name="cB")
                for kt in range(16):
                    kts = slice(kt * P, (kt + 1) * P)
                    psA = pps.tile([P, 1024], f32, tag="ps", name="ps")
                    psB = pps.tile([P, 1024], f32, tag="ps", name="ps")
                    for s2 in range(2):
                        ns = slice(s2 * 512, (s2 + 1) * 512)
                        qs = slice(q0 + s2 * 512, q0 + (s2 + 1) * 512)
                        nc.tensor.matmul(psA[:, ns],
                                         lhsT=r(kT[hp][0:64, kts]),
                                         rhs=r(qT[hp][0:64, qs]),
                                         start=True, stop=True)
                    for s2 in range(2):
                        ns = slice(s2 * 512, (s2 + 1) * 512)
                        qs = slice(q0 + s2 * 512, q0 + (s2 + 1) * 512)
                        nc.tensor.matmul(psB[:, ns],
                                         lhsT=r(kT[hp][64:128, kts]),
                                         rhs=r(qT[hp][64:128, qs]),
                                         start=True, stop=True)
                    eA = expp.tile([P, 1024], F32R, tag="e", name="e")
                    eB = expp.tile([P, 1024], F32R, tag="e", name="e")
                    nc.scalar.activation(out=eA, in_=psA, func=Exp, scale=SCALE)
                    nc.scalar.activation(out=eB, in_=psB, func=Exp, scale=SCALE)
                    for s2 in range(2):
                        ns = slice(s2 * 512, (s2 + 1) * 512)
                        nc.tensor.matmul(cA[:, ns],
                                         lhsT=r(v[kt][:, 2 * hp, :]),
                                         rhs=r(eA[:, ns]),
                                         start=(kt == 0), stop=(kt == 15))
                    for s2 in range(2):
                        ns = slice(s2 * 512, (s2 + 1) * 512)
                        nc.tensor.matmul(cB[:, ns],
                                         lhsT=r(v[kt][:, 2 * hp + 1, :]),
                                         rhs=r(eB[:, ns]),
                                         start=(kt == 0), stop=(kt == 15))
                # normalize: ctx[0:64] * (1 / ctx[64]) , broadcast across rows
                for c_ps, half in ((cA, 0), (cB, 1)):
                    rec = pn.tile([1, 1024], f32, tag="rec", name="rec")
                    nc.vector.reciprocal(out=rec, in_=c_ps[DK:DK + 1, :])
                    bc = pn.tile([DK, 1024], f32, tag="bc", name="bc")
                    nc.gpsimd.partition_broadcast(bc, rec)
                    nc.vector.tensor_mul(
                        out=cx[hp][half * DK:(half + 1) * DK, q0:q0 + 1024],
                        in0=c_ps[0:DK, :], in1=bc)
            # output projection for this q-half
            for qb2 in range(2):
                qs = slice(q0 + qb2 * 512, q0 + (qb2 + 1) * 512)
                for m in range(8):
                    po = pps.tile([P, 512], f32, tag="ps", name="ps")
                    for ct in range(4):
                        nc.tensor.matmul(po,
                                         lhsT=r(wo_sb[:, ct, m * P:(m + 1) * P]),
                                         rhs=r(cx[ct][:, qs]),
                                         start=(ct == 0), stop=(ct == 3))
                    osb = posb.tile([P, 512], f32, tag="o", name="o")
                    nc.vector.tensor_copy(out=osb, in_=po)
                    nc.sync.dma_start(out=t["outT"][m * P:(m + 1) * P, qs], in_=osb)


def _build():
    if "nc" in _cache:
        return _cache["nc"]
    from contextlib import ExitStack

    import concourse.tile as tile
    from concourse import bacc, mybir

    f32 = mybir.dt.float32
    nc = bacc.Bacc("TRN2", target_bir_lowering=False, debug=False,
                   num_devices=N_CORES)
    t = {}
    f32r = mybir.dt.float32r
    for name, shape in [("xqT", (D, S)), ("xkT", (D, S)), ("xvT", (D, S)),
                        ("wqT", (D, GS)), ("wkT", (D, GS)), ("wvT", (D, GS)),
                        ("woT", (GS, D))]:
        t[name] = nc.dram_tensor(name, shape, f32r, kind="ExternalInput").ap()
    for name, shape in [("bq", (GS,)), ("bk", (GS,)), ("bvb", (P, GS))]:
        t[name] = nc.dram_tensor(name, shape, f32, kind="ExternalInput").ap()
    t["outT"] = nc.dram_tensor("outT", (D, S), f32, kind="ExternalOutput").ap()

    with tile.TileContext(nc) as tc:
        with ExitStack() as ctx:
            _emit(ctx, tc, nc, t)
    nc.compile()
    _cache["nc"] = nc
    return nc


def _shard(query, key, value, wq, bq, wk, bk, wv, bv, wo):
    c32 = lambda a: np.ascontiguousarray(np.asarray(a, dtype=np.float32))
    xT = [{
        "xqT": c32(query[b].T), "xkT": c32(key[b].T), "xvT": c32(value[b].T),
    } for b in range(B)]
    wg = []
    for g in range(2):
        rows = slice(g * GS, (g + 1) * GS)
        wg.append({
            "wqT": c32(wq[rows, :].T), "wkT": c32(wk[rows, :].T),
            "wvT": c32(wv[rows, :].T), "woT": c32(wo[:, rows].T),
            "bq": c32(bq[rows]), "bk": c32(bk[rows]),
            "bvb": c32(np.broadcast_to(np.asarray(bv)[rows][None, :], (P, GS))),
        })
    return [dict(**xT[c // 2], **wg[c % 2]) for c in range(N_CORES)]


def kernel(query, key, value, mask, wq, bq, wk, bk, wv, bv, wo, bo):
    mask = np.asarray(mask)
    if not mask.all():
        return _numpy_fallback(query, key, value, mask, wq, bq, wk, bk,
                               wv, bv, wo, bo)

    from concourse.bass_utils import run_bass_kernel_spmd

    nc = _build()
    in_maps = _shard(query, key, value, wq, bq, wk, bk, wv, bv, wo)

    if PROFILE_DIR is not None:
        import contextlib
        try:
            from trn_agent_boot.trn_boot import _ntff_profile_via_ctypes
            hook = _ntff_profile_via_ctypes("/opt/axon/libaxon_pjrt.so")
            hctx = hook(PROFILE_DIR, [0]) if hook else contextlib.nullcontext()
        except Exception:
            hctx = contextlib.nullcontext()
        with hctx:
            res = run_bass_kernel_spmd(nc, in_maps, core_ids=list(range(N_CORES)))
    else:
        res = run_bass_kernel_spmd(nc, in_maps, core_ids=list(range(N_CORES)))

    bo = np.asarray(bo, dtype=np.float32)
    out = np.empty((B, S, D), dtype=np.float32)
    for b in range(B):
        p0 = res.results[2 * b]["outT"]
        p1 = res.results[2 * b + 1]["outT"]
        out[b] = p0.T + p1.T + bo
    return out


def _numpy_fallback(query, key, value, mask, wq, bq, wk, bk, wv, bv, wo, bo):
    q = np.asarray(query, np.float32) @ np.asarray(wq).T + np.asarray(bq)
    k = np.asarray(key, np.float32) @ np.asarray(wk).T + np.asarray(bk)
    v = np.asarray(value, np.float32) @ np.asarray(wv).T + np.asarray(bv)

    def heads(x):
        return x.reshape(B, S, H, DK).transpose(0, 2, 1, 3)

    q, k, v = heads(q), heads(k), heads(v)
    s = np.einsum("bhqd,bhkd->bhqk", q, k) / np.sqrt(DK).astype(np.float32)
    s = np.where(mask, s, np.float32(-1e9))
    s -= s.max(-1, keepdims=True)
    e = np.exp(s)
    a = e / e.sum(-1, keepdims=True)
    ctx = np.einsum("bhqk,bhkd->bhqd", a, v)
    ctx = ctx.transpose(0, 2, 1, 3).reshape(B, S, D)
    return (ctx @ np.asarray(wo).T + np.asarray(bo)).astype(np.float32)


# revision 7
# speedup vs baseline: 1.1495x; 1.1495x over previous
"""Multi-head attention (B=4, S=2048, D=1024, H=16) on 8 Trainium2 NeuronCores.

Sharding: (batch, head-group) tensor parallel — core c handles batch b = c//2
and head group g = c%2 (8 heads x 64 dims = 512 proj dims). Each core:
  qT/kT = Wg @ x.T         (projection, transposed layout [dims, tokens])
  v     = x @ Wv_g.T       (natural layout [tokens, dims], +ones column)
  per head: scoresT[k,q] = kh-tiles.T x qh-tiles; exp on ACT (scale=1/8 folded);
  ctxT[65,q] = [vh|1].T @ exp  (row 64 = softmax denominator, fused);
  normalize ctxT by reciprocal of row 64 (gpsimd partition-broadcast + DVE mul);
  outT_partial[1024, q] = woT_g.T @ ctxT.
Host sums the two head-group partials per batch and adds bo.

DT selects the matmul input dtype: "bf16" (1 cyc/row PE) or "f32r"
(fp32 data at 2 cyc/row). PSUM accumulation is fp32 either way.
"""

import numpy as np

B, S, D, H, DK = 4, 2048, 1024, 16, 64
HPC = 8          # heads per core
GS = HPC * DK    # 512 projection dims per head group
N_CORES = 8
P = 128
SCALE = 0.125    # 1 / sqrt(DK)

DT = "bf16"            # "bf16" | "f32r"
PROFILE_DIR = None     # if set, wrap device run with NTFF profile hook

_cache = {}


def _emit(ctx, tc, nc, t, MMDT):
    from concourse import mybir

    f32 = mybir.dt.float32
    Exp = mybir.ActivationFunctionType.Exp

    # ---- resident tiles --------------------------------------------------
    res = ctx.enter_context(tc.tile_pool(name="res", bufs=1))
    kT = [res.tile([P, S], MMDT, tag=f"kT{i}", name=f"kT{i}") for i in range(4)]
    qT = [res.tile([P, S], MMDT, tag=f"qT{i}", name=f"qT{i}") for i in range(4)]
    v = [res.tile([P, HPC, DK + 1], MMDT, tag=f"v{i}", name=f"v{i}")
         for i in range(16)]

    bias = ctx.enter_context(tc.tile_pool(name="bias", bufs=1))
    bq_sb = bias.tile([P, 4], f32, tag="bq", name="bq")
    nc.sync.dma_start(out=bq_sb, in_=t["bq"].rearrange("(m p) -> p m", p=P))
    bk_sb = bias.tile([P, 4], f32, tag="bk", name="bk")
    nc.sync.dma_start(out=bk_sb, in_=t["bk"].rearrange("(m p) -> p m", p=P))
    bvb_sb = bias.tile([P, GS], f32, tag="bvb", name="bvb")
    nc.sync.dma_start(out=bvb_sb, in_=t["bvb"])

    # ones column for the fused softmax-denominator row of the AV matmul
    # (memset can't target fp32r; memset an f32 staging tile and DVE-copy,
    # which performs the dtype conversion on write)
    ones_sb = bias.tile([P, HPC, 1], f32, tag="ones", name="ones")
    nc.vector.memset(ones_sb, 1.0)
    for i in range(16):
        nc.vector.tensor_copy(out=v[i][:, :, DK:DK + 1], in_=ones_sb)

    # ---- phase 1: projections (order V, K, Q; weights double-buffered) ---
    with tc.tile_pool(name="w", bufs=2) as wp, \
         tc.tile_pool(name="xc", bufs=2) as xp, \
         tc.tile_pool(name="pp", bufs=2, space="PSUM") as pp:
        wv_sb = wp.tile([P, 8, GS], MMDT, tag="w", name="wv")
        nc.sync.dma_start(out=wv_sb, in_=t["wvT"].rearrange("(k p) m -> p k m", p=P))
        wk_sb = wp.tile([P, 8, GS], MMDT, tag="w", name="wk")
        nc.sync.dma_start(out=wk_sb, in_=t["wkT"].rearrange("(k p) m -> p k m", p=P))

        # V projection: natural layout, [128 tokens, 512 dims] tiles
        bvb_h = bvb_sb.rearrange("p (h d) -> p h d", h=HPC)
        for c in range(4):
            xt = xp.tile([P, 8, 512], MMDT, tag="x", name="x")
            nc.sync.dma_start(
                out=xt,
                in_=t["xvT"][:, c * 512:(c + 1) * 512].rearrange(
                    "(k p) n -> p k n", p=P),
            )
            for tt in range(4):
                ps = pp.tile([P, GS], f32, tag="ps", name="ps")
                for k in range(8):
                    nc.tensor.matmul(
                        ps,
                        lhsT=xt[:, k, tt * P:(tt + 1) * P],
                        rhs=wv_sb[:, k, :],
                        start=(k == 0), stop=(k == 7),
                    )
                nc.vector.tensor_add(
                    out=v[c * 4 + tt][:, :, 0:DK],
                    in0=ps.rearrange("p (h d) -> p h d", h=HPC),
                    in1=bvb_h)

        # K then Q projections: transposed layout, [128 dims, 512 token] tiles
        wq_sb = wp.tile([P, 8, GS], MMDT, tag="w", name="wq")
        nc.sync.dma_start(out=wq_sb, in_=t["wqT"].rearrange("(k p) m -> p k m", p=P))

        for xT_dram, w_sb, b_sb, out_tiles in (
                (t["xkT"], wk_sb, bk_sb, kT),
                (t["xqT"], wq_sb, bq_sb, qT)):
            for c in range(4):
                xt = xp.tile([P, 8, 512], MMDT, tag="x", name="x")
                nc.sync.dma_start(
                    out=xt,
                    in_=xT_dram[:, c * 512:(c + 1) * 512].rearrange(
                        "(k p) n -> p k n", p=P),
                )
                for m in range(4):
                    ps = pp.tile([P, 512], f32, tag="ps", name="ps")
                    for k in range(8):
                        nc.tensor.matmul(
                            ps,
                            lhsT=w_sb[:, k, m * P:(m + 1) * P],
                            rhs=xt[:, k, :],
                            start=(k == 0), stop=(k == 7),
                        )
                    nc.vector.tensor_scalar_add(
                        out=out_tiles[m][:, c * 512:(c + 1) * 512],
                        in0=ps, scalar1=b_sb[:, m:m + 1])

    # ---- phases 2+3: attention + output projection ----------------------
    res2 = ctx.enter_context(tc.tile_pool(name="res2", bufs=1))
    cx = [res2.tile([P, S], MMDT, tag=f"cx{i}", name=f"cx{i}") for i in range(4)]

    with tc.tile_pool(name="wo", bufs=1) as wop, \
         tc.tile_pool(name="pps", bufs=2, space="PSUM") as pps, \
         tc.tile_pool(name="pcx", bufs=1, space="PSUM") as pcx, \
         tc.tile_pool(name="expp", bufs=4) as expp, \
         tc.tile_pool(name="norm", bufs=2) as pn, \
         tc.tile_pool(name="osb", bufs=3) as posb:
        wo_sb = wop.tile([P, 4, D], MMDT, tag="wo", name="wo")
        nc.sync.dma_start(out=wo_sb, in_=t["woT"].rearrange("(c p) m -> p c m", p=P))

        for qh in range(2):
            q0 = qh * 1024
            for hp in range(4):
                cA = pcx.tile([DK + 1, 1024], f32, tag="cA", name="cA")
                cB = pcx.tile([DK + 1, 1024], f32, tag="cB", name="cB")
                for kt in range(16):
                    kts = slice(kt * P, (kt + 1) * P)
                    psA = pps.tile([P, 1024], f32, tag="ps", name="ps")
                    psB = pps.tile([P, 1024], f32, tag="ps", name="ps")
                    for s2 in range(2):
                        ns = slice(s2 * 512, (s2 + 1) * 512)
                        qs = slice(q0 + s2 * 512, q0 + (s2 + 1) * 512)
                        nc.tensor.matmul(psA[:, ns],
                                         lhsT=kT[hp][0:64, kts],
                                         rhs=qT[hp][0:64, qs],
                                         start=True, stop=True)
                    for s2 in range(2):
                        ns = slice(s2 * 512, (s2 + 1) * 512)
                        qs = slice(q0 + s2 * 512, q0 + (s2 + 1) * 512)
                        nc.tensor.matmul(psB[:, ns],
                                         lhsT=kT[hp][64:128, kts],
                                         rhs=qT[hp][64:128, qs],
                                         start=True, stop=True)
                    eA = expp.tile([P, 1024], MMDT, tag="e", name="e")
                    eB = expp.tile([P, 1024], MMDT, tag="e", name="e")
                    nc.scalar.activation(out=eA, in_=psA, func=Exp, scale=SCALE)
                    nc.scalar.activation(out=eB, in_=psB, func=Exp, scale=SCALE)
                    for s2 in range(2):
                        ns = slice(s2 * 512, (s2 + 1) * 512)
                        nc.tensor.matmul(cA[:, ns],
                                         lhsT=v[kt][:, 2 * hp, :],
                                         rhs=eA[:, ns],
                                         start=(kt == 0), stop=(kt == 15))
                    for s2 in range(2):
                        ns = slice(s2 * 512, (s2 + 1) * 512)
                        nc.tensor.matmul(cB[:, ns],
                                         lhsT=v[kt][:, 2 * hp + 1, :],
                                         rhs=eB[:, ns],
                                         start=(kt == 0), stop=(kt == 15))
                # Copy ctx+denominator out of PSUM fast (frees the psum bank
                # for the next head pair), then normalize off the critical
                # path: ctx[0:64] * 1/ctx[64], broadcast down partitions.
                for c_ps, half in ((cA, 0), (cB, 1)):
                    stg = pn.tile([DK + 1, 1024], f32, tag="stg", name="stg")
                    nc.vector.tensor_copy(out=stg, in_=c_ps)
                    rec = pn.tile([1, 1024], f32, tag="rec", name="rec")
                    nc.vector.reciprocal(out=rec, in_=stg[DK:DK + 1, :])
                    bc = pn.tile([DK, 1024], f32, tag="bc", name="bc")
                    nc.gpsimd.partition_broadcast(bc, rec)
                    nc.vector.tensor_mul(
                        out=cx[hp][half * DK:(half + 1) * DK, q0:q0 + 1024],
                        in0=stg[0:DK, :], in1=bc)
            # output projection for this q-half
            for qb2 in range(2):
                qs = slice(q0 + qb2 * 512, q0 + (qb2 + 1) * 512)
                for m in range(8):
                    po = pps.tile([P, 512], f32, tag="ps", name="po")
                    for ct in range(4):
                        nc.tensor.matmul(po,
                                         lhsT=wo_sb[:, ct, m * P:(m + 1) * P],
                                         rhs=cx[ct][:, qs],
                                         start=(ct == 0), stop=(ct == 3))
                    osb = posb.tile([P, 512], f32, tag="o", name="o")
                    nc.vector.tensor_copy(out=osb, in_=po)
                    nc.sync.dma_start(out=t["outT"][m * P:(m + 1) * P, qs], in_=osb)


def _build(dt_name):
    key = ("nc", dt_name)
    if key in _cache:
        return _cache[key]
    from contextlib import ExitStack

    import concourse.tile as tile
    from concourse import bacc, mybir

    f32 = mybir.dt.float32
    MMDT = mybir.dt.bfloat16 if dt_name == "bf16" else mybir.dt.float32r
    nc = bacc.Bacc("TRN2", target_bir_lowering=False, debug=False,
                   num_devices=N_CORES)
    t = {}
    for name, shape in [("xqT", (D, S)), ("xkT", (D, S)), ("xvT", (D, S)),
                        ("wqT", (D, GS)), ("wkT", (D, GS)), ("wvT", (D, GS)),
                        ("woT", (GS, D))]:
        t[name] = nc.dram_tensor(name, shape, MMDT, kind="ExternalInput").ap()
    for name, shape in [("bq", (GS,)), ("bk", (GS,)), ("bvb", (P, GS))]:
        t[name] = nc.dram_tensor(name, shape, f32, kind="ExternalInput").ap()
    t["outT"] = nc.dram_tensor("outT", (D, S), f32, kind="ExternalOutput").ap()

    with tile.TileContext(nc) as tc:
        with ExitStack() as ctx:
            _emit(ctx, tc, nc, t, MMDT)
    nc.compile()
    _cache[key] = nc
    return nc


def _np_mmdt(dt_name):
    if dt_name == "bf16":
        import ml_dtypes
        return ml_dtypes.bfloat16
    return np.float32


def _shard(dt_name, query, key, value, wq, bq, wk, bk, wv, bv, wo):
    mdt = _np_mmdt(dt_name)
    cm = lambda a: np.ascontiguousarray(np.asarray(a, dtype=np.float32).astype(mdt))
    c32 = lambda a: np.ascontiguousarray(np.asarray(a, dtype=np.float32))
    xT = [{
        "xqT": cm(np.asarray(query)[b].T), "xkT": cm(np.asarray(key)[b].T),
        "xvT": cm(np.asarray(value)[b].T),
    } for b in range(B)]
    wg = []
    for g in range(2):
        rows = slice(g * GS, (g + 1) * GS)
        wg.append({
            "wqT": cm(np.asarray(wq)[rows, :].T), "wkT": cm(np.asarray(wk)[rows, :].T),
            "wvT": cm(np.asarray(wv)[rows, :].T), "woT": cm(np.asarray(wo)[:, rows].T),
            "bq": c32(np.asarray(bq)[rows]), "bk": c32(np.asarray(bk)[rows]),
            "bvb": c32(np.broadcast_to(np.asarray(bv)[rows][None, :], (P, GS))),
        })
    return [dict(**xT[c // 2], **wg[c % 2]) for c in range(N_CORES)]


def kernel(query, key, value, mask, wq, bq, wk, bk, wv, bv, wo, bo):
    mask = np.asarray(mask)
    if not mask.all():
        return _numpy_fallback(query, key, value, mask, wq, bq, wk, bk,
                               wv, bv, wo, bo)

    from concourse.bass_utils import run_bass_kernel_spmd

    nc = _build(DT)
    in_maps = _shard(DT, query, key, value, wq, bq, wk, bk, wv, bv, wo)

    if PROFILE_DIR is not None:
        import contextlib
        try:
            from trn_agent_boot.trn_boot import _ntff_profile_via_ctypes
            hook = _ntff_profile_via_ctypes("/opt/axon/libaxon_pjrt.so")
            hctx = hook(PROFILE_DIR, [0]) if hook else contextlib.nullcontext()
        except Exception:
            hctx = contextlib.nullcontext()
        with hctx:
            res = run_bass_kernel_spmd(nc, in_maps, core_ids=list(range(N_CORES)))
    else:
        res = run_bass_kernel_spmd(nc, in_maps, core_ids=list(range(N_CORES)))

    bo = np.asarray(bo, dtype=np.float32)
    out = np.empty((B, S, D), dtype=np.float32)
    for b in range(B):
        p0 = res.results[2 * b]["outT"]
        p1 = res.results[2 * b + 1]["outT"]
        out[b] = p0.T + p1.T + bo
    return out


def _numpy_fallback(query, key, value, mask, wq, bq, wk, bk, wv, bv, wo, bo):
    q = np.asarray(query, np.float32) @ np.asarray(wq).T + np.asarray(bq)
    k = np.asarray(key, np.float32) @ np.asarray(wk).T + np.asarray(bk)
    v = np.asarray(value, np.float32) @ np.asarray(wv).T + np.asarray(bv)

    def heads(x):
        return x.reshape(B, S, H, DK).transpose(0, 2, 1, 3)

    q, k, v = heads(q), heads(k), heads(v)
    s = np.einsum("bhqd,bhkd->bhqk", q, k) / np.sqrt(DK).astype(np.float32)
    s = np.where(mask, s, np.float32(-1e9))
    s -= s.max(-1, keepdims=True)
    e = np.exp(s)
    a = e / e.sum(-1, keepdims=True)
    ctx = np.einsum("bhqk,bhkd->bhqd", a, v)
    ctx = ctx.transpose(0, 2, 1, 3).reshape(B, S, D)
    return (ctx @ np.asarray(wo).T + np.asarray(bo)).astype(np.float32)
